# revision 1
# baseline (speedup 1.0000x reference)
"""Trainium2 Bass kernel for the Balle PDF-estimator (per-channel tiny MLP).

p(x) = CDF(x+0.5) - CDF(x-0.5), CDF = sigmoid(L3(g2(L2(g1(L1(g0(L0(x))))))))
with per-channel affine layers L_i (weights softplus(h_i), bias b_i) and gates
g_i(t) = t + tanh(a_i) * tanh(t).

Strategy (pure data parallel over B, 8 cores x 2 batches):
 - channel groups [42,42,42,42,24], planar components-on-partitions [3G, S]
   (row r*G+c = component r of channel c; x replicated 3x by DMA)
 - layer0 folded into ACT: tau0 = tanh(w0*x + beta0) via per-partition scale/bias
 - block-"diagonal" f32r matmuls on PE; all biases folded into ACT bias vectors
 - gates z = v + g (*) tanh(v) on DVE scalar_tensor_tensor
 - last gate folded into PE: v3 = (W2@W3).z1 + (g2*W3).tau2 with zero-padded
   M=2G weights so both branches accumulate into one [2G,S] psum at base 0
 - final subtract via PE with [I; -I] weights, DVE copies psum->sbuf
"""

import sys

if "/opt/trn_rl_repo" not in sys.path:
    sys.path.insert(0, "/opt/trn_rl_repo")

import numpy as np

import concourse.bacc as bacc
import concourse.bass as bass
import concourse.tile as tile
from concourse import mybir
from concourse.bass_utils import run_bass_kernel_spmd

F32 = mybir.dt.float32
F32R = mybir.dt.float32r
AF = mybir.ActivationFunctionType
OP = mybir.AluOpType

B, C, H, W_, R = 16, 192, 128, 128, 3
E = H * W_                      # 16384
NCORES = 8
B_LOC = B // NCORES             # 2
GROUPS = [42, 42, 42, 42, 24]   # channels per matmul group (3G <= 128)
GOFF = [0, 42, 84, 126, 168]
NG = len(GROUPS)
GMAX = max(GROUPS)
GMIN = min(GROUPS)
PMAX = 3 * GMAX                 # 126
S = 1024                        # strip width (elements of E per tile)
NSTRIP = E // S
MM_N = 512                      # psum-bank-limited matmul free dim
NSLICE = S // MM_N

# wmat column layout (fixed offsets sized for G=42):
W1X_C, G1_C, W2_C, W32_C, G3_C = 0, PMAX, 2 * PMAX, 3 * PMAX, 4 * PMAX
WMAT_COLS = 5 * PMAX            # 630
# pvec column layout
PV_W0, PV_B0P, PV_B0M, PV_B1P, PV_B1M, PV_B2P, PV_B2M, PV_G1, PV_B3 = range(9)
PVEC_COLS = 16

_NC_CACHE = {}


def _build(b_loc=B_LOC, nstrip=NSTRIP):
    nc = bacc.Bacc("TRN2", target_bir_lowering=False, debug=False)
    x_d = nc.dram_tensor("x", [b_loc, C, nstrip * S], F32R, kind="ExternalInput")
    wmat_d = nc.dram_tensor("wmat", [NG, PMAX, WMAT_COLS], F32R, kind="ExternalInput")
    isub_d = nc.dram_tensor("isub", [2 * GMAX, GMAX + GMIN], F32R,
                            kind="ExternalInput")
    pvec_d = nc.dram_tensor("pvec", [NG, PMAX, PVEC_COLS], F32, kind="ExternalInput")
    p_d = nc.dram_tensor("p", [b_loc, C, nstrip * S], F32, kind="ExternalOutput")

    with tile.TileContext(nc) as tc:
        with (
            tc.tile_pool(name="wpool", bufs=1) as wpool,
            tc.tile_pool(name="xp", bufs=4) as xp,
            tc.tile_pool(name="tau0", bufs=6) as tau0p_,
            tc.tile_pool(name="tau1", bufs=6) as tau1p_,
            tc.tile_pool(name="tau2", bufs=6) as tau2p_,
            tc.tile_pool(name="z1", bufs=6) as z1p_,
            tc.tile_pool(name="sig", bufs=4) as sigp_,
            tc.tile_pool(name="outp", bufs=4) as outp_,
            tc.tile_pool(name="ps12", bufs=3, space="PSUM") as ps12,
            tc.tile_pool(name="ps3", bufs=1, space="PSUM") as ps3,
        ):
            # resident weights / param vectors.  isub columns: [I42/-I42 | I24/-I24]
            isub_t = wpool.tile([2 * GMAX, GMAX + GMIN], F32R)
            nc.sync.dma_start(out=isub_t, in_=isub_d[:, :])
            w_t, pv_t = [], []
            for gi in range(NG):
                wt = wpool.tile([PMAX, WMAT_COLS], F32R, tag=f"w{gi}", name=f"w{gi}")
                nc.sync.dma_start(out=wt, in_=wmat_d[gi])
                pv = wpool.tile([PMAX, PVEC_COLS], F32, tag=f"pv{gi}", name=f"pv{gi}")
                nc.sync.dma_start(out=pv, in_=pvec_d[gi])
                w_t.append(wt)
                pv_t.append(pv)

            for b in range(b_loc):
                for gi in range(NG):
                    G = GROUPS[gi]
                    P3 = 3 * G
                    c0 = GOFF[gi]
                    wt = w_t[gi]
                    pv = pv_t[gi]

                    def col(c, n=P3):
                        return pv[:n, c : c + 1]

                    w1x = wt[:P3, W1X_C : W1X_C + P3]
                    g1m = wt[:P3, G1_C : G1_C + P3]
                    w2m = wt[:P3, W2_C : W2_C + P3]
                    w32p = wt[:P3, W32_C + G : W32_C + 3 * G]
                    w32m = wt[:P3, W32_C : W32_C + 2 * G]
                    g3p = wt[:P3, G3_C + G : G3_C + 3 * G]
                    g3mm = wt[:P3, G3_C : G3_C + 2 * G]
                    if G == GMAX:
                        isub_g = isub_t[: 2 * G, :G]
                    else:
                        isub_g = isub_t[: 2 * G, GMAX : GMAX + G]

                    for so in range(0, nstrip, 2):
                      # x + tau0 batched over 2 strips (SBUF-src ACT, FD=2S)
                      e00 = so * S
                      x_t = xp.tile([PMAX, 2 * S], F32R, tag="x", name="x_t")
                      src = x_d[b, c0 : c0 + G, e00 : e00 + 2 * S]
                      for r in range(3):
                          nc.sync.dma_start(
                              out=x_t[r * G : (r + 1) * G, :], in_=src
                          )
                      t0 = {}
                      for sg, bcol in ((+1, PV_B0P), (-1, PV_B0M)):
                          t0[sg] = tau0p_.tile([PMAX, 2 * S], F32R, tag="tau0",
                                               name="t0")
                          nc.scalar.activation(
                              t0[sg][:P3], x_t[:P3], AF.Tanh,
                              bias=col(bcol), scale=col(PV_W0),
                          )
                      for si in range(so, so + 2):
                        e0 = si * S
                        lo = (si - so) * S

                        # v1 = W1X.x + G1.tau0 ; tau1 ; z1 = v1 + g1*tau1
                        z1 = {}
                        for sg, bcol in ((+1, PV_B1P), (-1, PV_B1M)):
                            v1 = ps12.tile([PMAX, S], F32, tag="ps12", name="v1")
                            for k in range(NSLICE):
                                sl = slice(k * MM_N, (k + 1) * MM_N)
                                slx = slice(lo + k * MM_N, lo + (k + 1) * MM_N)
                                nc.tensor.matmul(
                                    v1[:P3, sl], w1x, x_t[:P3, slx],
                                    start=True, stop=False,
                                )
                                nc.tensor.matmul(
                                    v1[:P3, sl], g1m, t0[sg][:P3, slx],
                                    start=False, stop=True,
                                )
                            t1 = tau1p_.tile([PMAX, S], F32, tag="tau1", name="t1")
                            nc.scalar.activation(
                                t1[:P3], v1[:P3], AF.Tanh, bias=col(bcol)
                            )
                            z1[sg] = z1p_.tile([PMAX, S], F32R, tag="z1", name="z1t")
                            nc.vector.scalar_tensor_tensor(
                                z1[sg][:P3], t1[:P3], col(PV_G1), v1[:P3],
                                OP.mult, OP.add,
                            )

                        # v2 = W2.z1 ; tau2
                        t2 = {}
                        for sg, bcol in ((+1, PV_B2P), (-1, PV_B2M)):
                            v2 = ps12.tile([PMAX, S], F32, tag="ps12", name="v2")
                            for k in range(NSLICE):
                                sl = slice(k * MM_N, (k + 1) * MM_N)
                                nc.tensor.matmul(
                                    v2[:P3, sl], w2m, z1[sg][:P3, sl],
                                    start=True, stop=True,
                                )
                            t2[sg] = tau2p_.tile([PMAX, S], F32R, tag="tau2",
                                                 name="t2")
                            nc.scalar.activation(
                                t2[sg][:P3], v2[:P3], AF.Tanh, bias=col(bcol)
                            )

                        # v3(+/-) packed [2G,S]: rows 0:G = plus, G:2G = minus
                        v3 = ps3.tile([2 * GMAX, S], F32, tag="ps3", name="v3")
                        for k in range(NSLICE):
                            sl = slice(k * MM_N, (k + 1) * MM_N)
                            nc.tensor.matmul(
                                v3[: 2 * G, sl], w32p, z1[+1][:P3, sl],
                                start=True, stop=False,
                            )
                            nc.tensor.matmul(
                                v3[: 2 * G, sl], g3p, t2[+1][:P3, sl],
                                start=False, stop=False,
                            )
                            nc.tensor.matmul(
                                v3[: 2 * G, sl], w32m, z1[-1][:P3, sl],
                                start=False, stop=False,
                            )
                            nc.tensor.matmul(
                                v3[: 2 * G, sl], g3mm, t2[-1][:P3, sl],
                                start=False, stop=True,
                            )
                        sig = sigp_.tile([2 * GMAX, S], F32R, tag="sig",
                                         name="sig")
                        nc.scalar.activation(
                            sig[: 2 * G], v3[: 2 * G], AF.Sigmoid,
                            bias=pv[: 2 * G, PV_B3 : PV_B3 + 1],
                        )
                        # p = sig[:G] - sig[G:2G] via PE with [I; -I] weights;
                        # reuse v3's banks (its data is dead after sigma reads it)
                        for k in range(NSLICE):
                            sl = slice(k * MM_N, (k + 1) * MM_N)
                            nc.tensor.matmul(
                                v3[:G, sl], isub_g, sig[: 2 * G, sl],
                                start=True, stop=True, skip_group_check=True,
                            )
                        p_t = outp_.tile([GMAX, S], F32, tag="out", name="p_t")
                        nc.vector.tensor_copy(p_t[:G], v3[:G])
                        nc.sync.dma_start(
                            out=p_d[b, c0 : c0 + G, e0 : e0 + S], in_=p_t[:G]
                        )
    nc.compile()
    return nc


def _host_params(h0, h1, h2, h3, a0, a1, a2, b0, b1, b2, b3):
    """Fold weights/biases on host (float64) into device tensors."""
    f64 = np.float64
    sp = lambda v: np.log1p(np.exp(v.astype(f64)))
    W0 = sp(h0)[:, 0, :]          # [C,R]
    W1 = sp(h1)                   # [C,R,R]  W1[c,d,r]
    W2 = sp(h2)
    W3 = sp(h3)[:, :, 0]          # [C,R]
    g0 = np.tanh(a0.astype(f64))
    g1 = np.tanh(a1.astype(f64))
    g2 = np.tanh(a2.astype(f64))

    wmat = np.zeros((NG, PMAX, WMAT_COLS), np.float32)
    pvec = np.zeros((NG, PMAX, PVEC_COLS), np.float32)

    W32 = np.einsum("cdr,cr->cd", W2, W3)   # [C,R]
    G3 = W3 * g2                            # [C,R]

    be0 = {+1: b0.astype(f64) + 0.5 * W0, -1: b0.astype(f64) - 0.5 * W0}
    be1 = {s: b1.astype(f64) + np.einsum("cdr,cd->cr", W1, be0[s]) for s in be0}
    be2 = {s: b2.astype(f64) + np.einsum("cdr,cd->cr", W2, be1[s]) for s in be0}
    be3 = {s: b3[:, 0].astype(f64) + np.einsum("cd,cd->c", W3, be2[s]) for s in be0}

    for gi in range(NG):
        G = GROUPS[gi]
        cs = slice(GOFF[gi], GOFF[gi] + G)
        for ci, c in enumerate(range(GOFF[gi], GOFF[gi] + G)):
            for d in range(R):
                row = d * G + ci
                for r in range(R):
                    wmat[gi, row, W1X_C + r * G + ci] = W1[c, d, r] * W0[c, d]
                    wmat[gi, row, G1_C + r * G + ci] = W1[c, d, r] * g0[c, d]
                    wmat[gi, row, W2_C + r * G + ci] = W2[c, d, r]
                wmat[gi, row, W32_C + G + ci] = W32[c, d]
                wmat[gi, row, G3_C + G + ci] = G3[c, d]
        # per-partition vectors, planar: row r*G+ci = component r of channel c
        for vcol, arr in [
            (PV_W0, W0), (PV_B0P, be0[+1]), (PV_B0M, be0[-1]),
            (PV_B1P, be1[+1]), (PV_B1M, be1[-1]),
            (PV_B2P, be2[+1]), (PV_B2M, be2[-1]), (PV_G1, g1),
        ]:
            pvec[gi, : 3 * G, vcol] = arr[cs].T.reshape(-1)
        pvec[gi, :G, PV_B3] = be3[+1][cs]
        pvec[gi, G : 2 * G, PV_B3] = be3[-1][cs]
    return wmat, pvec


def _host_isub():
    isub = np.zeros((2 * GMAX, GMAX + GMIN), np.float32)
    isub[:GMAX, :GMAX] = np.eye(GMAX, dtype=np.float32)
    isub[GMAX:, :GMAX] = -np.eye(GMAX, dtype=np.float32)
    isub[:GMIN, GMAX:] = np.eye(GMIN, dtype=np.float32)
    isub[GMIN : 2 * GMIN, GMAX:] = -np.eye(GMIN, dtype=np.float32)
    return isub


def kernel(x_tilde, h0, h1, h2, h3, a0, a1, a2, b0, b1, b2, b3, _trace=False):
    key = "full"
    if key not in _NC_CACHE:
        _NC_CACHE[key] = _build()
    nc = _NC_CACHE[key]

    wmat, pvec = _host_params(h0, h1, h2, h3, a0, a1, a2, b0, b1, b2, b3)
    isub = _host_isub()
    x = np.ascontiguousarray(x_tilde.astype(np.float32).reshape(B, C, E))
    in_maps = [
        {"x": x[i * B_LOC : (i + 1) * B_LOC], "wmat": wmat, "pvec": pvec,
         "isub": isub}
        for i in range(NCORES)
    ]
    kw = {}
    if _trace:
        kw = dict(trace=True)
    res = run_bass_kernel_spmd(nc, in_maps, core_ids=list(range(NCORES)), **kw)
    p = np.concatenate([res.results[i]["p"] for i in range(NCORES)], axis=0)
    out = p.reshape(B, C, H, W_).astype(np.float32)
    if _trace:
        return out, res
    return out

